# revision 18
# baseline (speedup 1.0000x reference)
"""Direct-Form-II biquad (order-2 IIR) over [B=64, T=262144, 1] on 8 trn2 cores.

Algorithm
---------
The recurrence
    y[t] = b0 x[t] + b1 x[t-1] + b2 x[t-2] - a1 y[t-1] - a2 y[t-2]
is a linear time-invariant filter whose impulse response g decays
geometrically (|poles| < 1 for the sampled coefficients), so to fp32
precision the IIR equals a short FIR: y = conv(x, g[:K]).

On device the FIR is computed with the tensor engine in overlap-save form.
Per sequence, x is laid out in SBUF as [128 partitions, 2048] with partition
p holding x[p*2048 : (p+1)*2048] (contiguous DMA). Each 128x128 tile of that
layout holds 128 chunks (partitions = chunk index c = p*16 + f1, free =
within-chunk time j). Tiles are PE-transposed so j lands on partitions, then
one matmul per tile, with the transposed tile as the stationary operand and a
fused [A^T | B^T] Toeplitz coefficient block as the moving operand, produces
the within-chunk FIR term (A-half) and the spill-over into the next chunk
(B-half). ys[f1] = A(f1) + B(f1-1) is assembled during PSUM evacuation:
an A-copy (ACT/DVE) plus a read-modify-write B-add (DVE) — PSUM has a single
DVE read port, so the two PSUM halves are never read by one instruction.

Sharding: pure data parallelism, batch 64 -> 8 sequences per core.
"""

import os
from contextlib import ExitStack

import numpy as np

_B, _T = 64, 262144
_NCORES = 8
_S = _B // _NCORES          # sequences per core
_P = 128                    # partitions / chunk length
_F = _T // _P               # 2048 free columns per sequence
_NT = _F // _P              # 16 tiles per sequence

# 'fp16'  : half-precision datapath — halves HBM traffic, full PE rate
# 'fp32'  : exact fp32 matmuls (4 cycles/row on PE)
# 'f32r'  : rounded fp32 (12-bit mantissa) matmuls at full PE rate
_MODE = os.environ.get("BIQUAD_MODE", "fp16")

_runner_cache = {}


def _impulse_response(b0, b1, b2, a1, a2, n):
    """Float64 impulse response of the reference recurrence."""
    g = np.zeros(n, dtype=np.float64)
    v0 = 0.0
    v1 = 0.0
    for t in range(n):
        xt = 1.0 if t == 0 else 0.0
        out = xt * b0 + v0
        v0_new = xt * b1 + v1 - out * a1
        v1_new = xt * b2 - out * a2
        v0, v1 = v0_new, v1_new
        g[t] = out
    return g


def _coef_block(g, kb):
    """[128, 128 + kb] moving operand: columns = output offset i.

    A^T[j, i] = g[i - j]          (within-chunk taps, i in [0,128))
    B^T[j, i] = g[i + 128 - j]    (taps reaching one chunk back, i in [0,kb))
    """
    K = len(g)
    A = np.zeros((_P, _P), dtype=np.float64)
    Bm = np.zeros((_P, kb), dtype=np.float64)
    for j in range(_P):
        for i in range(_P):
            if 0 <= i - j < K:
                A[j, i] = g[i - j]
        for i in range(kb):
            k = i + _P - j
            if 0 <= k < K:
                Bm[j, i] = g[k]
    return np.concatenate([A, Bm], axis=1).astype(np.float32)


def _build_program(mode, kb, repeat=1):
    from concourse import bacc, mybir, tile

    nc = bacc.Bacc("TRN2", target_bir_lowering=False, debug=False)
    f32 = mybir.dt.float32
    f16 = mybir.dt.float16
    if mode == "fp16":
        cdt = f16       # compute dtype (stationary/moving operands)
        iodt = f16      # HBM I/O dtype
    elif mode == "f32r":
        cdt = mybir.dt.float32r
        iodt = f32
    else:
        cdt = f32
        iodt = f32

    NC = _P + kb                      # moving operand width
    SLOT = 256 if NC <= 256 else 512  # psum slot stride (bank-crossing safe)
    x_d = nc.dram_tensor("x", [_S, _P, _F], iodt, kind="ExternalInput")
    coef_d = nc.dram_tensor("coef", [_P, NC], iodt, kind="ExternalInput")
    id_d = nc.dram_tensor("ident", [_P, _P], iodt, kind="ExternalInput")
    y_d = nc.dram_tensor("y", [_S, _P, _F], iodt, kind="ExternalOutput")

    XT_SLOTS = _NT + 1                # 16 transposed tiles + shifted m1 tile

    with tile.TileContext(nc) as tc, ExitStack() as ctx:
        cpool = ctx.enter_context(tc.tile_pool(name="consts", bufs=1))
        xpool = ctx.enter_context(tc.tile_pool(name="xin", bufs=3))
        xtpool = ctx.enter_context(tc.tile_pool(name="xt", bufs=2))
        ypool = ctx.enter_context(tc.tile_pool(name="yout", bufs=2))
        ptp = ctx.enter_context(tc.tile_pool(name="pt", bufs=2, space="PSUM"))
        pyp = ctx.enter_context(tc.tile_pool(name="py", bufs=4, space="PSUM"))

        id_sb = cpool.tile([_P, _P], iodt)
        nc.sync.dma_start(id_sb[:], id_d.ap())
        coef_sb = cpool.tile([_P, NC], iodt)
        nc.sync.dma_start(coef_sb[:], coef_d.ap())
        if mode == "f32r":
            coef_c = cpool.tile([_P, NC], cdt)
            nc.vector.tensor_copy(coef_c[:], coef_sb[:])
        else:
            coef_c = coef_sb

        def load_stage(s):
            # load x[s] as [128, 2048] in two half-loads so the first
            # transpose group starts after ~half the transfer
            xs = xpool.tile([_P, _F], iodt)
            nc.sync.dma_start(xs[:, 0 : _F // 2], x_d.ap()[s][:, 0 : _F // 2])
            nc.sync.dma_start(xs[:, _F // 2 : _F], x_d.ap()[s][:, _F // 2 : _F])
            return xs

        def transpose_stage(xs):
            # PE transposes, 8 per PSUM bank (fp16: 2 KiB/partition).
            xt = xtpool.tile([_P, XT_SLOTS * _P], cdt)
            # element-indexed view for the m1 boundary ops (gpsimd can't
            # address float32r; fp16/f32 are fine natively)
            xt32 = xt[:].bitcast(f32) if mode == "f32r" else xt[:]
            for gidx in range(2):
                ptile = ptp.tile([_P, 8 * _P], iodt)
                for q in range(8):
                    f1 = gidx * 8 + q
                    nc.tensor.transpose(
                        ptile[:, q * _P : (q + 1) * _P],
                        xs[:, f1 * _P : (f1 + 1) * _P],
                        id_sb[:],
                    )
                # dense contiguous evacuation (gpsimd cannot read PSUM)
                eng = nc.vector.tensor_copy if gidx == 0 else nc.scalar.copy
                eng(
                    xt[:, gidx * 8 * _P : (gidx + 1) * 8 * _P],
                    ptile[:, 0 : 8 * _P],
                )

            # m1 boundary tile: m1[col p] = tile15[col p-1], col 0 = 0
            m1 = _NT * _P
            nc.gpsimd.memset(xt32[:, m1 : m1 + 1], 0.0)
            nc.gpsimd.tensor_copy(
                xt32[:, m1 + 1 : m1 + _P],
                xt32[:, 15 * _P : 16 * _P - 1],
            )
            return xt

        def fir_stage(s, xt):
            # FIR matmuls, PSUM-accumulated B-half, dense evacuation.
            # Per output tile c: psum[c] = xt[c] @ A (start) then += xt[c-1]
            # @ B (accumulate).  Tiles descend 15..0 so consecutive matmuls
            # share stationary operands; groups of 4 tiles per PSUM bank.
            ys = ypool.tile([_P, _F], iodt)
            evac = [nc.scalar.copy, nc.vector.tensor_copy] * 2
            for gi, hi in enumerate([15, 11, 7, 3]):
                lo = hi - 3
                pt_ = pyp.tile([_P, 4 * _P], f32, tag="py")
                for c in range(hi, lo - 1, -1):
                    col = (c - lo) * _P
                    nc.tensor.matmul(
                        pt_[:, col : col + _P],
                        xt[:, c * _P : (c + 1) * _P],
                        coef_c[:, 0:_P],
                        start=True,
                        stop=False,
                    )
                    prev = _NT if c == 0 else c - 1   # m1 tile for c == 0
                    nc.tensor.matmul(
                        pt_[:, col : col + kb],
                        xt[:, prev * _P : (prev + 1) * _P],
                        coef_c[:, _P : _P + kb],
                        start=False,
                        stop=True,
                    )
                evac[gi](
                    ys[:, lo * _P : (hi + 1) * _P],
                    pt_[:, 0 : 4 * _P],
                )
                # stream each finished half of y out immediately, on
                # separate queues, so the output DMA doesn't trail compute
                if hi == 11:
                    nc.gpsimd.dma_start(
                        y_d.ap()[s][:, _F // 2 : _F], ys[:, _F // 2 : _F]
                    )
                elif hi == 3:
                    nc.sync.dma_start(
                        y_d.ap()[s][:, 0 : _F // 2], ys[:, 0 : _F // 2]
                    )

        from contextlib import nullcontext
        loop_ctx = tc.For_i(0, repeat, 1) if repeat > 1 else nullcontext()
        with loop_ctx:
            # Software-pipelined across sequences: the PE transposes of
            # sequence s+1 are emitted BEFORE the FIR matmuls of sequence
            # s, so the PE never stalls on the cross-engine xt evacuation
            # round-trip (stalls reset the PE DVFS ramp to half clock).
            xts = [None] * _S
            xts[0] = transpose_stage(load_stage(0))
            for s in range(_S):
                if s + 1 < _S:
                    xts[s + 1] = transpose_stage(load_stage(s + 1))
                fir_stage(s, xts[s])
                xts[s] = None

    nc.compile()
    return nc


def _make_runner(mode, kb, repeat=1):
    """Compile the bass program and wrap it in a cached shard_map'd jit."""
    import jax
    import numpy as _np
    from jax.sharding import Mesh, PartitionSpec
    from jax.experimental.shard_map import shard_map
    from concourse import bass2jax, mybir

    nc = _build_program(mode, kb, repeat)

    io_np = np.float16 if mode == "fp16" else np.float32

    if os.environ.get("BIQUAD_SIM") == "1":
        def run_sim(x_all, coef):
            from concourse import bass_interp
            y_all = np.zeros_like(x_all)
            ident = np.eye(_P, dtype=io_np)
            ncs = int(os.environ.get("BIQUAD_SIM_CORES", str(_NCORES)))
            for c in range(ncs):
                sim = bass_interp.CoreSim(nc)
                sim.tensor("x")[:] = x_all[c * _S : (c + 1) * _S]
                sim.tensor("coef")[:] = coef
                sim.tensor("ident")[:] = ident
                sim.simulate()
                y_all[c * _S : (c + 1) * _S] = sim.tensor("y")
            return y_all
        return run_sim

    bass2jax.install_neuronx_cc_hook()

    partition_name = (
        nc.partition_id_tensor.name if nc.partition_id_tensor else None
    )
    in_names, out_names, out_avals = [], [], []
    for alloc in nc.m.functions[0].allocations:
        if not isinstance(alloc, mybir.MemoryLocationSet):
            continue
        name = alloc.memorylocations[0].name
        if alloc.kind == "ExternalInput":
            if name != partition_name:
                in_names.append(name)
        elif alloc.kind == "ExternalOutput":
            out_names.append(name)
            out_avals.append(
                jax.core.ShapedArray(
                    tuple(alloc.tensor_shape), mybir.dt.np(alloc.dtype)
                )
            )
    n_params = len(in_names)
    in_names.extend(out_names)
    if partition_name is not None:
        in_names.append(partition_name)

    def _body(*args):
        operands = list(args)
        if partition_name is not None:
            operands.append(bass2jax.partition_id_tensor())
        outs = bass2jax._bass_exec_p.bind(
            *operands,
            out_avals=tuple(out_avals),
            in_names=tuple(in_names),
            out_names=tuple(out_names),
            lowering_input_output_aliases=(),
            sim_require_finite=True,
            sim_require_nnan=True,
            nc=nc,
        )
        return tuple(outs)

    devices = jax.devices()[:_NCORES]
    mesh = Mesh(_np.asarray(devices), ("core",))
    n_outs = len(out_names)
    in_specs = (PartitionSpec("core"),) * (n_params + n_outs)
    out_specs = (PartitionSpec("core"),) * n_outs
    sharded = jax.jit(
        shard_map(
            _body, mesh=mesh, in_specs=in_specs, out_specs=out_specs,
            check_rep=False,
        ),
        keep_unused=True,
    )

    name_to_idx = {n: i for i, n in enumerate(in_names[:n_params])}
    ident = np.eye(_P, dtype=io_np)

    def run_hw(x_all, coef):
        # x_all: [64, 128, 2048] fp32; returns y_all same shape
        per_core_ins = {
            "x": x_all.reshape(_NCORES * _S, _P, _F),
            "coef": np.concatenate([coef] * _NCORES, axis=0),
            "ident": np.concatenate([ident] * _NCORES, axis=0),
        }
        args = [None] * n_params
        for n, i in name_to_idx.items():
            args[i] = per_core_ins[n]
        zeros = [
            np.zeros((_NCORES * a.shape[0], *a.shape[1:]), a.dtype)
            for a in out_avals
        ]
        outs = sharded(*args, *zeros)
        y_idx = out_names.index("y")
        return np.asarray(outs[y_idx]).reshape(_B, _P, _F)

    run_hw.sharded = sharded
    run_hw.meta = (in_names, out_names, out_avals, n_params, name_to_idx, ident)
    run_hw.nc = nc

    def make_chain():
        """Jit that runs the kernel k (runtime scalar) times back-to-back on
        device, feeding y back as x — for timing (marginal cost per step ≈
        one on-device execution). fori_loop keeps the bass_exec custom call
        appearing exactly once in the module (hook limitation), and a
        runtime k avoids recompiling per chain length."""
        x_idx = name_to_idx["x"]
        y_idx = out_names.index("y")

        def chained(k, *args):
            args = list(args)

            def body(_, x):
                a = list(args)
                a[x_idx] = x
                return _body(*a)[y_idx]

            y = jax.lax.fori_loop(0, k, body, args[x_idx])
            return (y,)

        return jax.jit(
            shard_map(
                chained, mesh=mesh,
                in_specs=(PartitionSpec(),) + in_specs,
                out_specs=(PartitionSpec("core"),),
                check_rep=False,
            ),
            keep_unused=True,
        )

    run_hw.make_chain = make_chain
    return run_hw


def _get_runner(mode, kb, repeat=1):
    key = (mode, kb, repeat, os.environ.get("BIQUAD_SIM") == "1")
    if key not in _runner_cache:
        _runner_cache[key] = _make_runner(mode, kb, repeat)
    return _runner_cache[key]


def _prepare(b0, b1, b2, a1, a2):
    """Impulse response, truncation length, coefficient block."""
    g = _impulse_response(b0, b1, b2, a1, a2, 2 * _P)
    mag = np.abs(g)
    scale = mag.max() + 1e-300
    sig = np.nonzero(mag > 1e-9 * scale)[0]
    K = int(sig[-1]) + 1 if len(sig) else 1
    if K > _P:
        raise ValueError(
            f"impulse response needs {K} taps (> {_P}); filter too close "
            "to instability for the truncated-FIR kernel"
        )
    kb = max(32, ((K + 15) // 16) * 16)   # B-half width, 16-col aligned
    if _MODE == "f32r":
        kb = _P                            # keep N >= 256 for full-rate f32r
    coef = _coef_block(g[: _P + kb], kb)
    return coef, kb


def kernel(x, b0, b1, b2, a1, a2):
    assert x.shape == (_B, _T, 1), x.shape
    coef, kb = _prepare(
        float(b0[0]), float(b1[0]), float(b2[0]), float(a1[0]), float(a2[0])
    )
    run = _get_runner(_MODE, kb)
    io_np = np.float16 if _MODE == "fp16" else np.float32
    x_all = np.ascontiguousarray(x, dtype=io_np).reshape(_B, _P, _F)
    y_all = run(x_all, coef.astype(io_np))
    return y_all.reshape(_B, _T, 1).astype(np.float32)



# revision 20
# speedup vs baseline: 1.0185x; 1.0185x over previous
"""Direct-Form-II biquad (order-2 IIR) over [B=64, T=262144, 1] on 8 trn2 cores.

Algorithm
---------
The recurrence
    y[t] = b0 x[t] + b1 x[t-1] + b2 x[t-2] - a1 y[t-1] - a2 y[t-2]
is a linear time-invariant filter whose impulse response g decays
geometrically (|poles| < 1 for the sampled coefficients), so to fp32
precision the IIR equals a short FIR: y = conv(x, g[:K]).

On device the FIR is computed with the tensor engine in overlap-save form.
Per sequence, x is laid out in SBUF as [128 partitions, 2048] with partition
p holding x[p*2048 : (p+1)*2048] (contiguous DMA). Each 128x128 tile of that
layout holds 128 chunks (partitions = chunk index c = p*16 + f1, free =
within-chunk time j). Tiles are PE-transposed so j lands on partitions, then
one matmul per tile, with the transposed tile as the stationary operand and a
fused [A^T | B^T] Toeplitz coefficient block as the moving operand, produces
the within-chunk FIR term (A-half) and the spill-over into the next chunk
(B-half). ys[f1] = A(f1) + B(f1-1) is assembled during PSUM evacuation:
an A-copy (ACT/DVE) plus a read-modify-write B-add (DVE) — PSUM has a single
DVE read port, so the two PSUM halves are never read by one instruction.

Sharding: pure data parallelism, batch 64 -> 8 sequences per core.
"""

import os
from contextlib import ExitStack

import numpy as np

_B, _T = 64, 262144
_NCORES = 8
_S = _B // _NCORES          # sequences per core
_P = 128                    # partitions / chunk length
_F = _T // _P               # 2048 free columns per sequence
_NT = _F // _P              # 16 tiles per sequence

# 'fp16'  : half-precision datapath — halves HBM traffic, full PE rate
# 'fp32'  : exact fp32 matmuls (4 cycles/row on PE)
# 'f32r'  : rounded fp32 (12-bit mantissa) matmuls at full PE rate
_MODE = os.environ.get("BIQUAD_MODE", "fp16")

_runner_cache = {}


def _impulse_response(b0, b1, b2, a1, a2, n):
    """Float64 impulse response of the reference recurrence."""
    g = np.zeros(n, dtype=np.float64)
    v0 = 0.0
    v1 = 0.0
    for t in range(n):
        xt = 1.0 if t == 0 else 0.0
        out = xt * b0 + v0
        v0_new = xt * b1 + v1 - out * a1
        v1_new = xt * b2 - out * a2
        v0, v1 = v0_new, v1_new
        g[t] = out
    return g


def _coef_block(g, kb):
    """[128, 128 + kb] moving operand: columns = output offset i.

    A^T[j, i] = g[i - j]          (within-chunk taps, i in [0,128))
    B^T[j, i] = g[i + 128 - j]    (taps reaching one chunk back, i in [0,kb))
    """
    K = len(g)
    A = np.zeros((_P, _P), dtype=np.float64)
    Bm = np.zeros((_P, kb), dtype=np.float64)
    for j in range(_P):
        for i in range(_P):
            if 0 <= i - j < K:
                A[j, i] = g[i - j]
        for i in range(kb):
            k = i + _P - j
            if 0 <= k < K:
                Bm[j, i] = g[k]
    return np.concatenate([A, Bm], axis=1).astype(np.float32)


def _build_program(mode, kb, repeat=1):
    from concourse import bacc, mybir, tile

    nc = bacc.Bacc("TRN2", target_bir_lowering=False, debug=False)
    f32 = mybir.dt.float32
    f16 = mybir.dt.float16
    if mode == "fp16":
        cdt = f16       # compute dtype (stationary/moving operands)
        iodt = f16      # HBM I/O dtype
    elif mode == "f32r":
        cdt = mybir.dt.float32r
        iodt = f32
    else:
        cdt = f32
        iodt = f32

    NC = _P + kb                      # moving operand width
    SLOT = 256 if NC <= 256 else 512  # psum slot stride (bank-crossing safe)
    x_d = nc.dram_tensor("x", [_S, _P, _F], iodt, kind="ExternalInput")
    coef_d = nc.dram_tensor("coef", [_P, NC], iodt, kind="ExternalInput")
    id_d = nc.dram_tensor("ident", [_P, _P], iodt, kind="ExternalInput")
    y_d = nc.dram_tensor("y", [_S, _P, _F], iodt, kind="ExternalOutput")

    XT_SLOTS = _NT + 1                # 16 transposed tiles + shifted m1 tile

    with tile.TileContext(nc) as tc, ExitStack() as ctx:
        cpool = ctx.enter_context(tc.tile_pool(name="consts", bufs=1))
        xpool = ctx.enter_context(tc.tile_pool(name="xin", bufs=3))
        xtpool = ctx.enter_context(tc.tile_pool(name="xt", bufs=2))
        ypool = ctx.enter_context(tc.tile_pool(name="yout", bufs=2))
        ptp = ctx.enter_context(tc.tile_pool(name="pt", bufs=2, space="PSUM"))
        pyp = ctx.enter_context(tc.tile_pool(name="py", bufs=4, space="PSUM"))

        id_sb = cpool.tile([_P, _P], iodt)
        nc.sync.dma_start(id_sb[:], id_d.ap())
        coef_sb = cpool.tile([_P, NC], iodt)
        nc.sync.dma_start(coef_sb[:], coef_d.ap())
        if mode == "f32r":
            coef_c = cpool.tile([_P, NC], cdt)
            nc.vector.tensor_copy(coef_c[:], coef_sb[:])
        else:
            coef_c = coef_sb

        def load_stage(s):
            # load x[s] as [128, 2048] in two half-loads so the first
            # transpose group starts after ~half the transfer
            xs = xpool.tile([_P, _F], iodt)
            nc.sync.dma_start(xs[:, 0 : _F // 2], x_d.ap()[s][:, 0 : _F // 2])
            nc.sync.dma_start(xs[:, _F // 2 : _F], x_d.ap()[s][:, _F // 2 : _F])
            return xs

        def transpose_stage(xs):
            # PE transposes, 8 per PSUM bank (fp16: 2 KiB/partition).
            xt = xtpool.tile([_P, XT_SLOTS * _P], cdt)
            # element-indexed view for the m1 boundary ops (gpsimd can't
            # address float32r; fp16/f32 are fine natively)
            xt32 = xt[:].bitcast(f32) if mode == "f32r" else xt[:]
            for gidx in range(2):
                ptile = ptp.tile([_P, 8 * _P], iodt)
                for q in range(8):
                    f1 = gidx * 8 + q
                    nc.tensor.transpose(
                        ptile[:, q * _P : (q + 1) * _P],
                        xs[:, f1 * _P : (f1 + 1) * _P],
                        id_sb[:],
                    )
                # dense contiguous evacuation (gpsimd cannot read PSUM)
                eng = nc.vector.tensor_copy if gidx == 0 else nc.scalar.copy
                eng(
                    xt[:, gidx * 8 * _P : (gidx + 1) * 8 * _P],
                    ptile[:, 0 : 8 * _P],
                )

            # m1 boundary tile: m1[col p] = tile15[col p-1], col 0 = 0
            m1 = _NT * _P
            nc.gpsimd.memset(xt32[:, m1 : m1 + 1], 0.0)
            nc.gpsimd.tensor_copy(
                xt32[:, m1 + 1 : m1 + _P],
                xt32[:, 15 * _P : 16 * _P - 1],
            )
            return xt

        def fir_stage(s, xt):
            # FIR matmuls, PSUM-accumulated B-half, dense evacuation.
            # Per output tile c: psum[c] = xt[c] @ A (start) then += xt[c-1]
            # @ B (accumulate).  Tiles descend 15..0 so consecutive matmuls
            # share stationary operands; groups of 4 tiles per PSUM bank.
            ys = ypool.tile([_P, _F], iodt)
            evac = [nc.scalar.copy, nc.vector.tensor_copy] * 2
            for gi, hi in enumerate([15, 11, 7, 3]):
                lo = hi - 3
                pt_ = pyp.tile([_P, 4 * _P], f32, tag="py")
                for c in range(hi, lo - 1, -1):
                    col = (c - lo) * _P
                    nc.tensor.matmul(
                        pt_[:, col : col + _P],
                        xt[:, c * _P : (c + 1) * _P],
                        coef_c[:, 0:_P],
                        start=True,
                        stop=False,
                    )
                    prev = _NT if c == 0 else c - 1   # m1 tile for c == 0
                    nc.tensor.matmul(
                        pt_[:, col : col + kb],
                        xt[:, prev * _P : (prev + 1) * _P],
                        coef_c[:, _P : _P + kb],
                        start=False,
                        stop=True,
                    )
                evac[gi](
                    ys[:, lo * _P : (hi + 1) * _P],
                    pt_[:, 0 : 4 * _P],
                )
                # stream each finished half of y out immediately, on
                # separate queues, so the output DMA doesn't trail compute
                if hi == 11:
                    nc.gpsimd.dma_start(
                        y_d.ap()[s][:, _F // 2 : _F], ys[:, _F // 2 : _F]
                    )
                elif hi == 3:
                    nc.sync.dma_start(
                        y_d.ap()[s][:, 0 : _F // 2], ys[:, 0 : _F // 2]
                    )

        from contextlib import nullcontext
        loop_ctx = tc.For_i(0, repeat, 1) if repeat > 1 else nullcontext()
        with loop_ctx:
            # Software-pipelined across sequences: the PE transposes of
            # sequence s+1 are emitted BEFORE the FIR matmuls of sequence
            # s, so the PE never stalls on the cross-engine xt evacuation
            # round-trip (stalls reset the PE DVFS ramp to half clock).
            xts = [None] * _S
            xts[0] = transpose_stage(load_stage(0))
            for s in range(_S):
                if s + 1 < _S:
                    xts[s + 1] = transpose_stage(load_stage(s + 1))
                fir_stage(s, xts[s])
                xts[s] = None

    nc.compile()
    return nc


def _make_runner(mode, kb, repeat=1):
    """Compile the bass program and wrap it in a cached shard_map'd jit."""
    import jax
    import numpy as _np
    from jax.sharding import Mesh, PartitionSpec
    from jax.experimental.shard_map import shard_map
    from concourse import bass2jax, mybir

    nc = _build_program(mode, kb, repeat)

    io_np = np.float16 if mode == "fp16" else np.float32

    if os.environ.get("BIQUAD_SIM") == "1":
        def run_sim(x_all, coef):
            from concourse import bass_interp
            y_all = np.zeros_like(x_all)
            ident = np.eye(_P, dtype=io_np)
            ncs = int(os.environ.get("BIQUAD_SIM_CORES", str(_NCORES)))
            for c in range(ncs):
                sim = bass_interp.CoreSim(nc)
                sim.tensor("x")[:] = x_all[c * _S : (c + 1) * _S]
                sim.tensor("coef")[:] = coef
                sim.tensor("ident")[:] = ident
                sim.simulate()
                y_all[c * _S : (c + 1) * _S] = sim.tensor("y")
            return y_all
        return run_sim

    bass2jax.install_neuronx_cc_hook()

    partition_name = (
        nc.partition_id_tensor.name if nc.partition_id_tensor else None
    )
    in_names, out_names, out_avals = [], [], []
    for alloc in nc.m.functions[0].allocations:
        if not isinstance(alloc, mybir.MemoryLocationSet):
            continue
        name = alloc.memorylocations[0].name
        if alloc.kind == "ExternalInput":
            if name != partition_name:
                in_names.append(name)
        elif alloc.kind == "ExternalOutput":
            out_names.append(name)
            out_avals.append(
                jax.core.ShapedArray(
                    tuple(alloc.tensor_shape), mybir.dt.np(alloc.dtype)
                )
            )
    n_params = len(in_names)
    in_names.extend(out_names)
    if partition_name is not None:
        in_names.append(partition_name)

    def _body(*args):
        operands = list(args)
        if partition_name is not None:
            operands.append(bass2jax.partition_id_tensor())
        outs = bass2jax._bass_exec_p.bind(
            *operands,
            out_avals=tuple(out_avals),
            in_names=tuple(in_names),
            out_names=tuple(out_names),
            lowering_input_output_aliases=(),
            sim_require_finite=True,
            sim_require_nnan=True,
            nc=nc,
        )
        return tuple(outs)

    devices = jax.devices()[:_NCORES]
    mesh = Mesh(_np.asarray(devices), ("core",))
    n_outs = len(out_names)
    in_specs = (PartitionSpec("core"),) * (n_params + n_outs)
    out_specs = (PartitionSpec("core"),) * n_outs
    sharded = jax.jit(
        shard_map(
            _body, mesh=mesh, in_specs=in_specs, out_specs=out_specs,
            check_rep=False,
        ),
        keep_unused=True,
    )

    name_to_idx = {n: i for i, n in enumerate(in_names[:n_params])}
    ident = np.eye(_P, dtype=io_np)

    def run_hw(x_all, coef):
        # x_all: [64, 128, 2048] fp32; returns y_all same shape
        per_core_ins = {
            "x": x_all.reshape(_NCORES * _S, _P, _F),
            "coef": np.concatenate([coef] * _NCORES, axis=0),
            "ident": np.concatenate([ident] * _NCORES, axis=0),
        }
        args = [None] * n_params
        for n, i in name_to_idx.items():
            args[i] = per_core_ins[n]
        zeros = [
            np.zeros((_NCORES * a.shape[0], *a.shape[1:]), a.dtype)
            for a in out_avals
        ]
        outs = sharded(*args, *zeros)
        y_idx = out_names.index("y")
        return np.asarray(outs[y_idx]).reshape(_B, _P, _F)

    run_hw.sharded = sharded
    run_hw.meta = (in_names, out_names, out_avals, n_params, name_to_idx, ident)
    run_hw.nc = nc

    def make_chain():
        """Jit that runs the kernel k (runtime scalar) times back-to-back on
        device, feeding y back as x — for timing (marginal cost per step ≈
        one on-device execution). fori_loop keeps the bass_exec custom call
        appearing exactly once in the module (hook limitation), and a
        runtime k avoids recompiling per chain length."""
        x_idx = name_to_idx["x"]
        y_idx = out_names.index("y")

        def chained(k, *args):
            args = list(args)

            def body(_, x):
                a = list(args)
                a[x_idx] = x
                return _body(*a)[y_idx]

            y = jax.lax.fori_loop(0, k, body, args[x_idx])
            return (y,)

        return jax.jit(
            shard_map(
                chained, mesh=mesh,
                in_specs=(PartitionSpec(),) + in_specs,
                out_specs=(PartitionSpec("core"),),
                check_rep=False,
            ),
            keep_unused=True,
        )

    run_hw.make_chain = make_chain
    return run_hw


def _get_runner(mode, kb, repeat=1):
    key = (mode, kb, repeat, os.environ.get("BIQUAD_SIM") == "1")
    if key not in _runner_cache:
        _runner_cache[key] = _make_runner(mode, kb, repeat)
    return _runner_cache[key]


def _prepare(b0, b1, b2, a1, a2):
    """Impulse response, truncation length, coefficient block."""
    g = _impulse_response(b0, b1, b2, a1, a2, 2 * _P)
    mag = np.abs(g)
    scale = mag.max() + 1e-300
    sig = np.nonzero(mag > 1e-9 * scale)[0]
    K = int(sig[-1]) + 1 if len(sig) else 1
    if K > _P:
        raise ValueError(
            f"impulse response needs {K} taps (> {_P}); filter too close "
            "to instability for the truncated-FIR kernel"
        )
    kb = max(32, ((K + 15) // 16) * 16)   # B-half width, 16-col aligned
    if _MODE == "f32r":
        kb = _P                            # keep N >= 256 for full-rate f32r
    coef = _coef_block(g[: _P + kb], kb)
    return coef, kb


def kernel(x, b0, b1, b2, a1, a2):
    assert x.shape == (_B, _T, 1), x.shape
    coef, kb = _prepare(
        float(b0[0]), float(b1[0]), float(b2[0]), float(a1[0]), float(a2[0])
    )
    run = _get_runner(_MODE, kb)
    io_np = np.float16 if _MODE == "fp16" else np.float32
    x_all = np.ascontiguousarray(x, dtype=io_np).reshape(_B, _P, _F)
    y_all = run(x_all, coef.astype(io_np))
    return y_all.reshape(_B, _T, 1).astype(np.float32)



# revision 23
# speedup vs baseline: 1.0270x; 1.0083x over previous
"""Direct-Form-II biquad (order-2 IIR) over [B=64, T=262144, 1] on 8 trn2 cores.

Algorithm
---------
The recurrence
    y[t] = b0 x[t] + b1 x[t-1] + b2 x[t-2] - a1 y[t-1] - a2 y[t-2]
is a linear time-invariant filter whose impulse response g decays
geometrically (|poles| < 1 for the sampled coefficients), so to fp32
precision the IIR equals a short FIR: y = conv(x, g[:K]).

On device the FIR is computed with the tensor engine in overlap-save form,
in an fp16 datapath (tolerance is 2e-2; fp16 contributes ~1e-3): fp16
halves HBM traffic — the hard floor for this memory-regime problem — and
runs the PE at 1 cycle/row instead of fp32's 4.

Per sequence, x is laid out in SBUF as [128 partitions, 2048] with partition
p holding x[p*2048 : (p+1)*2048] (contiguous DMA). Each 128x128 tile of that
layout holds 128 chunks (partitions = chunk index c = p*16 + f1, free =
within-chunk time j). Tiles are PE-transposed (8 per fp16 PSUM bank) so j
lands on partitions. Per output tile c, PSUM accumulates two matmuls with
the transposed tiles as stationary operands: xt[c] @ A^T (start) plus
xt[c-1] @ B^T (the spill-over taps reaching one chunk back), so evacuation
is a single dense PSUM->SBUF cast per 4-tile group (no read-modify-write).
Tiles descend 15..0 so consecutive matmuls share stationary operands; the
c=0 boundary uses an m1 tile (tile 15 shifted one column, built on gpsimd).

The PE stream is software-pipelined across sequences — the transposes of
sequence s+1 are emitted before the FIR matmuls of sequence s — so the PE
never stalls on the cross-engine xt-evacuation round-trip (stalls reset
the PE DVFS ramp to half clock). Each half of y streams out as soon as it
is evacuated, on separate DMA queues (sync + gpsimd), so the output DMA
does not trail the compute.

Sharding: pure data parallelism, batch 64 -> 8 sequences per core.
"""

import os
from contextlib import ExitStack

import numpy as np

_B, _T = 64, 262144
_NCORES = 8
_S = _B // _NCORES          # sequences per core
_P = 128                    # partitions / chunk length
_F = _T // _P               # 2048 free columns per sequence
_NT = _F // _P              # 16 tiles per sequence

# 'fp16'  : half-precision datapath — halves HBM traffic, full PE rate
# 'fp32'  : exact fp32 matmuls (4 cycles/row on PE)
# 'f32r'  : rounded fp32 (12-bit mantissa) matmuls at full PE rate
_MODE = os.environ.get("BIQUAD_MODE", "fp16")

_runner_cache = {}


def _impulse_response(b0, b1, b2, a1, a2, n):
    """Float64 impulse response of the reference recurrence."""
    g = np.zeros(n, dtype=np.float64)
    v0 = 0.0
    v1 = 0.0
    for t in range(n):
        xt = 1.0 if t == 0 else 0.0
        out = xt * b0 + v0
        v0_new = xt * b1 + v1 - out * a1
        v1_new = xt * b2 - out * a2
        v0, v1 = v0_new, v1_new
        g[t] = out
    return g


def _coef_block(g, kb):
    """[128, 128 + kb] moving operand: columns = output offset i.

    A^T[j, i] = g[i - j]          (within-chunk taps, i in [0,128))
    B^T[j, i] = g[i + 128 - j]    (taps reaching one chunk back, i in [0,kb))
    """
    K = len(g)
    A = np.zeros((_P, _P), dtype=np.float64)
    Bm = np.zeros((_P, kb), dtype=np.float64)
    for j in range(_P):
        for i in range(_P):
            if 0 <= i - j < K:
                A[j, i] = g[i - j]
        for i in range(kb):
            k = i + _P - j
            if 0 <= k < K:
                Bm[j, i] = g[k]
    return np.concatenate([A, Bm], axis=1).astype(np.float32)


def _build_program(mode, kb, repeat=1):
    from concourse import bacc, mybir, tile

    nc = bacc.Bacc("TRN2", target_bir_lowering=False, debug=False)
    f32 = mybir.dt.float32
    f16 = mybir.dt.float16
    if mode == "fp16":
        cdt = f16       # compute dtype (stationary/moving operands)
        iodt = f16      # HBM I/O dtype
    elif mode == "f32r":
        cdt = mybir.dt.float32r
        iodt = f32
    else:
        cdt = f32
        iodt = f32

    NC = _P + kb                      # moving operand width
    SLOT = 256 if NC <= 256 else 512  # psum slot stride (bank-crossing safe)
    x_d = nc.dram_tensor("x", [_S, _P, _F], iodt, kind="ExternalInput")
    coef_d = nc.dram_tensor("coef", [_P, NC], iodt, kind="ExternalInput")
    id_d = nc.dram_tensor("ident", [_P, _P], iodt, kind="ExternalInput")
    y_d = nc.dram_tensor("y", [_S, _P, _F], iodt, kind="ExternalOutput")

    XT_SLOTS = _NT + 1                # 16 transposed tiles + shifted m1 tile

    with tile.TileContext(nc) as tc, ExitStack() as ctx:
        cpool = ctx.enter_context(tc.tile_pool(name="consts", bufs=1))
        # all 8 sequences fit in SBUF in fp16 (4 MiB) — prefetch everything
        # upfront so the input DMA stream runs back-to-back at full rate
        xpool = ctx.enter_context(tc.tile_pool(name="xin", bufs=_S))
        xtpool = ctx.enter_context(tc.tile_pool(name="xt", bufs=2))
        ypool = ctx.enter_context(tc.tile_pool(name="yout", bufs=2))
        ptp = ctx.enter_context(tc.tile_pool(name="pt", bufs=2, space="PSUM"))
        pyp = ctx.enter_context(tc.tile_pool(name="py", bufs=4, space="PSUM"))

        id_sb = cpool.tile([_P, _P], iodt)
        nc.sync.dma_start(id_sb[:], id_d.ap())
        coef_sb = cpool.tile([_P, NC], iodt)
        nc.sync.dma_start(coef_sb[:], coef_d.ap())
        if mode == "f32r":
            coef_c = cpool.tile([_P, NC], cdt)
            nc.vector.tensor_copy(coef_c[:], coef_sb[:])
        else:
            coef_c = coef_sb

        def load_stage(s):
            # load x[s] as [128, 2048] in two half-loads so the first
            # transpose group starts after ~half the transfer
            xs = xpool.tile([_P, _F], iodt)
            nc.sync.dma_start(xs[:, 0 : _F // 2], x_d.ap()[s][:, 0 : _F // 2])
            nc.sync.dma_start(xs[:, _F // 2 : _F], x_d.ap()[s][:, _F // 2 : _F])
            return xs

        def transpose_stage(xs):
            # PE transposes, 8 per PSUM bank (fp16: 2 KiB/partition).
            xt = xtpool.tile([_P, XT_SLOTS * _P], cdt)
            # element-indexed view for the m1 boundary ops (gpsimd can't
            # address float32r; fp16/f32 are fine natively)
            xt32 = xt[:].bitcast(f32) if mode == "f32r" else xt[:]
            for gidx in range(2):
                ptile = ptp.tile([_P, 8 * _P], iodt)
                for q in range(8):
                    f1 = gidx * 8 + q
                    nc.tensor.transpose(
                        ptile[:, q * _P : (q + 1) * _P],
                        xs[:, f1 * _P : (f1 + 1) * _P],
                        id_sb[:],
                    )
                # dense contiguous evacuation (gpsimd cannot read PSUM)
                eng = nc.vector.tensor_copy if gidx == 0 else nc.scalar.copy
                eng(
                    xt[:, gidx * 8 * _P : (gidx + 1) * 8 * _P],
                    ptile[:, 0 : 8 * _P],
                )

            # m1 boundary tile: m1[col p] = tile15[col p-1], col 0 = 0
            m1 = _NT * _P
            nc.gpsimd.memset(xt32[:, m1 : m1 + 1], 0.0)
            nc.gpsimd.tensor_copy(
                xt32[:, m1 + 1 : m1 + _P],
                xt32[:, 15 * _P : 16 * _P - 1],
            )
            return xt

        def fir_stage(s, xt):
            # FIR matmuls, PSUM-accumulated B-half, dense evacuation.
            # Per output tile c: psum[c] = xt[c] @ A (start) then += xt[c-1]
            # @ B (accumulate).  Tiles descend 15..0 so consecutive matmuls
            # share stationary operands; groups of 4 tiles per PSUM bank.
            ys = ypool.tile([_P, _F], iodt)
            evac = [nc.scalar.copy, nc.vector.tensor_copy] * 2
            for gi, hi in enumerate([15, 11, 7, 3]):
                lo = hi - 3
                pt_ = pyp.tile([_P, 4 * _P], f32, tag="py")
                for c in range(hi, lo - 1, -1):
                    col = (c - lo) * _P
                    nc.tensor.matmul(
                        pt_[:, col : col + _P],
                        xt[:, c * _P : (c + 1) * _P],
                        coef_c[:, 0:_P],
                        start=True,
                        stop=False,
                    )
                    prev = _NT if c == 0 else c - 1   # m1 tile for c == 0
                    nc.tensor.matmul(
                        pt_[:, col : col + kb],
                        xt[:, prev * _P : (prev + 1) * _P],
                        coef_c[:, _P : _P + kb],
                        start=False,
                        stop=True,
                    )
                evac[gi](
                    ys[:, lo * _P : (hi + 1) * _P],
                    pt_[:, 0 : 4 * _P],
                )
                # stream each finished half of y out immediately, on
                # separate queues, so the output DMA doesn't trail compute
                if hi == 11:
                    nc.gpsimd.dma_start(
                        y_d.ap()[s][:, _F // 2 : _F], ys[:, _F // 2 : _F]
                    )
                elif hi == 3:
                    nc.sync.dma_start(
                        y_d.ap()[s][:, 0 : _F // 2], ys[:, 0 : _F // 2]
                    )

        from contextlib import nullcontext
        loop_ctx = tc.For_i(0, repeat, 1) if repeat > 1 else nullcontext()
        with loop_ctx:
            # All input DMAs issue upfront (back-to-back on the queue);
            # the PE stream is software-pipelined across sequences: the
            # transposes of sequence s+1 are emitted BEFORE the FIR
            # matmuls of sequence s, so the PE never stalls on the
            # cross-engine xt evacuation round-trip (stalls reset the PE
            # DVFS ramp to half clock).
            xss = [load_stage(s) for s in range(_S)]
            xts = [None] * _S
            xts[0] = transpose_stage(xss[0])
            for s in range(_S):
                if s + 1 < _S:
                    xts[s + 1] = transpose_stage(xss[s + 1])
                fir_stage(s, xts[s])
                xts[s] = None

    nc.compile()
    return nc


def _make_runner(mode, kb, repeat=1):
    """Compile the bass program and wrap it in a cached shard_map'd jit."""
    import jax
    import numpy as _np
    from jax.sharding import Mesh, PartitionSpec
    from jax.experimental.shard_map import shard_map
    from concourse import bass2jax, mybir

    nc = _build_program(mode, kb, repeat)

    io_np = np.float16 if mode == "fp16" else np.float32

    if os.environ.get("BIQUAD_SIM") == "1":
        def run_sim(x_all, coef):
            from concourse import bass_interp
            y_all = np.zeros_like(x_all)
            ident = np.eye(_P, dtype=io_np)
            ncs = int(os.environ.get("BIQUAD_SIM_CORES", str(_NCORES)))
            for c in range(ncs):
                sim = bass_interp.CoreSim(nc)
                sim.tensor("x")[:] = x_all[c * _S : (c + 1) * _S]
                sim.tensor("coef")[:] = coef
                sim.tensor("ident")[:] = ident
                sim.simulate()
                y_all[c * _S : (c + 1) * _S] = sim.tensor("y")
            return y_all
        return run_sim

    bass2jax.install_neuronx_cc_hook()

    partition_name = (
        nc.partition_id_tensor.name if nc.partition_id_tensor else None
    )
    in_names, out_names, out_avals = [], [], []
    for alloc in nc.m.functions[0].allocations:
        if not isinstance(alloc, mybir.MemoryLocationSet):
            continue
        name = alloc.memorylocations[0].name
        if alloc.kind == "ExternalInput":
            if name != partition_name:
                in_names.append(name)
        elif alloc.kind == "ExternalOutput":
            out_names.append(name)
            out_avals.append(
                jax.core.ShapedArray(
                    tuple(alloc.tensor_shape), mybir.dt.np(alloc.dtype)
                )
            )
    n_params = len(in_names)
    in_names.extend(out_names)
    if partition_name is not None:
        in_names.append(partition_name)

    def _body(*args):
        operands = list(args)
        if partition_name is not None:
            operands.append(bass2jax.partition_id_tensor())
        outs = bass2jax._bass_exec_p.bind(
            *operands,
            out_avals=tuple(out_avals),
            in_names=tuple(in_names),
            out_names=tuple(out_names),
            lowering_input_output_aliases=(),
            sim_require_finite=True,
            sim_require_nnan=True,
            nc=nc,
        )
        return tuple(outs)

    devices = jax.devices()[:_NCORES]
    mesh = Mesh(_np.asarray(devices), ("core",))
    n_outs = len(out_names)
    in_specs = (PartitionSpec("core"),) * (n_params + n_outs)
    out_specs = (PartitionSpec("core"),) * n_outs
    sharded = jax.jit(
        shard_map(
            _body, mesh=mesh, in_specs=in_specs, out_specs=out_specs,
            check_rep=False,
        ),
        keep_unused=True,
    )

    name_to_idx = {n: i for i, n in enumerate(in_names[:n_params])}
    ident = np.eye(_P, dtype=io_np)

    def run_hw(x_all, coef):
        # x_all: [64, 128, 2048] fp32; returns y_all same shape
        per_core_ins = {
            "x": x_all.reshape(_NCORES * _S, _P, _F),
            "coef": np.concatenate([coef] * _NCORES, axis=0),
            "ident": np.concatenate([ident] * _NCORES, axis=0),
        }
        args = [None] * n_params
        for n, i in name_to_idx.items():
            args[i] = per_core_ins[n]
        zeros = [
            np.zeros((_NCORES * a.shape[0], *a.shape[1:]), a.dtype)
            for a in out_avals
        ]
        outs = sharded(*args, *zeros)
        y_idx = out_names.index("y")
        return np.asarray(outs[y_idx]).reshape(_B, _P, _F)

    run_hw.sharded = sharded
    run_hw.meta = (in_names, out_names, out_avals, n_params, name_to_idx, ident)
    run_hw.nc = nc

    def make_chain():
        """Jit that runs the kernel k (runtime scalar) times back-to-back on
        device, feeding y back as x — for timing (marginal cost per step ≈
        one on-device execution). fori_loop keeps the bass_exec custom call
        appearing exactly once in the module (hook limitation), and a
        runtime k avoids recompiling per chain length."""
        x_idx = name_to_idx["x"]
        y_idx = out_names.index("y")

        def chained(k, *args):
            args = list(args)

            def body(_, x):
                a = list(args)
                a[x_idx] = x
                return _body(*a)[y_idx]

            y = jax.lax.fori_loop(0, k, body, args[x_idx])
            return (y,)

        return jax.jit(
            shard_map(
                chained, mesh=mesh,
                in_specs=(PartitionSpec(),) + in_specs,
                out_specs=(PartitionSpec("core"),),
                check_rep=False,
            ),
            keep_unused=True,
        )

    run_hw.make_chain = make_chain
    return run_hw


def _get_runner(mode, kb, repeat=1):
    key = (mode, kb, repeat, os.environ.get("BIQUAD_SIM") == "1")
    if key not in _runner_cache:
        _runner_cache[key] = _make_runner(mode, kb, repeat)
    return _runner_cache[key]


def _prepare(b0, b1, b2, a1, a2):
    """Impulse response, truncation length, coefficient block."""
    g = _impulse_response(b0, b1, b2, a1, a2, 2 * _P)
    mag = np.abs(g)
    scale = mag.max() + 1e-300
    sig = np.nonzero(mag > 1e-9 * scale)[0]
    K = int(sig[-1]) + 1 if len(sig) else 1
    if K > _P:
        raise ValueError(
            f"impulse response needs {K} taps (> {_P}); filter too close "
            "to instability for the truncated-FIR kernel"
        )
    kb = max(32, ((K + 15) // 16) * 16)   # B-half width, 16-col aligned
    if _MODE == "f32r":
        kb = _P                            # keep N >= 256 for full-rate f32r
    coef = _coef_block(g[: _P + kb], kb)
    return coef, kb


def kernel(x, b0, b1, b2, a1, a2):
    assert x.shape == (_B, _T, 1), x.shape
    coef, kb = _prepare(
        float(b0[0]), float(b1[0]), float(b2[0]), float(a1[0]), float(a2[0])
    )
    run = _get_runner(_MODE, kb)
    io_np = np.float16 if _MODE == "fp16" else np.float32
    x_all = np.ascontiguousarray(x, dtype=io_np).reshape(_B, _P, _F)
    y_all = run(x_all, coef.astype(io_np))
    return y_all.reshape(_B, _T, 1).astype(np.float32)



# revision 24
# speedup vs baseline: 1.0365x; 1.0092x over previous
"""Direct-Form-II biquad (order-2 IIR) over [B=64, T=262144, 1] on 8 trn2 cores.

Algorithm
---------
The recurrence
    y[t] = b0 x[t] + b1 x[t-1] + b2 x[t-2] - a1 y[t-1] - a2 y[t-2]
is a linear time-invariant filter whose impulse response g decays
geometrically (|poles| < 1 for the sampled coefficients), so to fp32
precision the IIR equals a short FIR: y = conv(x, g[:K]).

On device the FIR is computed with the tensor engine in overlap-save form,
in an fp16 datapath (tolerance is 2e-2; fp16 contributes ~1e-3): fp16
halves HBM traffic — the hard floor for this memory-regime problem — and
runs the PE at 1 cycle/row instead of fp32's 4.

Per sequence, x is laid out in SBUF as [128 partitions, 2048] with partition
p holding x[p*2048 : (p+1)*2048] (contiguous DMA). Each 128x128 tile of that
layout holds 128 chunks (partitions = chunk index c = p*16 + f1, free =
within-chunk time j). Tiles are PE-transposed (8 per fp16 PSUM bank) so j
lands on partitions. Per output tile c, PSUM accumulates two matmuls with
the transposed tiles as stationary operands: xt[c] @ A^T (start) plus
xt[c-1] @ B^T (the spill-over taps reaching one chunk back), so evacuation
is a single dense PSUM->SBUF cast per 4-tile group (no read-modify-write).
Tiles descend 15..0 so consecutive matmuls share stationary operands; the
c=0 boundary uses an m1 tile (tile 15 shifted one column, built on gpsimd).

The PE stream is software-pipelined across sequences — the transposes of
sequence s+1 are emitted before the FIR matmuls of sequence s — so the PE
never stalls on the cross-engine xt-evacuation round-trip (stalls reset
the PE DVFS ramp to half clock). Each half of y streams out as soon as it
is evacuated, on separate DMA queues (sync + gpsimd), so the output DMA
does not trail the compute.

Sharding: pure data parallelism, batch 64 -> 8 sequences per core.
"""

import os
from contextlib import ExitStack

import numpy as np

_B, _T = 64, 262144
_NCORES = 8
_S = _B // _NCORES          # sequences per core
_P = 128                    # partitions / chunk length
_F = _T // _P               # 2048 free columns per sequence
_NT = _F // _P              # 16 tiles per sequence

# 'fp16'  : half-precision datapath — halves HBM traffic, full PE rate
# 'fp32'  : exact fp32 matmuls (4 cycles/row on PE)
# 'f32r'  : rounded fp32 (12-bit mantissa) matmuls at full PE rate
_MODE = os.environ.get("BIQUAD_MODE", "fp16")

_runner_cache = {}


def _impulse_response(b0, b1, b2, a1, a2, n):
    """Float64 impulse response of the reference recurrence."""
    g = np.zeros(n, dtype=np.float64)
    v0 = 0.0
    v1 = 0.0
    for t in range(n):
        xt = 1.0 if t == 0 else 0.0
        out = xt * b0 + v0
        v0_new = xt * b1 + v1 - out * a1
        v1_new = xt * b2 - out * a2
        v0, v1 = v0_new, v1_new
        g[t] = out
    return g


def _coef_block(g, kb):
    """[128, 128 + kb] moving operand: columns = output offset i.

    A^T[j, i] = g[i - j]          (within-chunk taps, i in [0,128))
    B^T[j, i] = g[i + 128 - j]    (taps reaching one chunk back, i in [0,kb))
    """
    K = len(g)
    A = np.zeros((_P, _P), dtype=np.float64)
    Bm = np.zeros((_P, kb), dtype=np.float64)
    for j in range(_P):
        for i in range(_P):
            if 0 <= i - j < K:
                A[j, i] = g[i - j]
        for i in range(kb):
            k = i + _P - j
            if 0 <= k < K:
                Bm[j, i] = g[k]
    return np.concatenate([A, Bm], axis=1).astype(np.float32)


def _build_program(mode, kb, repeat=1):
    from concourse import bacc, mybir, tile

    nc = bacc.Bacc("TRN2", target_bir_lowering=False, debug=False)
    f32 = mybir.dt.float32
    f16 = mybir.dt.float16
    if mode == "fp16":
        cdt = f16       # compute dtype (stationary/moving operands)
        iodt = f16      # HBM I/O dtype
    elif mode == "f32r":
        cdt = mybir.dt.float32r
        iodt = f32
    else:
        cdt = f32
        iodt = f32

    NC = _P + kb                      # moving operand width
    SLOT = 256 if NC <= 256 else 512  # psum slot stride (bank-crossing safe)
    x_d = nc.dram_tensor("x", [_S, _P, _F], iodt, kind="ExternalInput")
    coef_d = nc.dram_tensor("coef", [_P, NC], iodt, kind="ExternalInput")
    id_d = nc.dram_tensor("ident", [_P, _P], iodt, kind="ExternalInput")
    y_d = nc.dram_tensor("y", [_S, _P, _F], iodt, kind="ExternalOutput")

    XT_SLOTS = _NT + 1                # 16 transposed tiles + shifted m1 tile

    with tile.TileContext(nc) as tc, ExitStack() as ctx:
        cpool = ctx.enter_context(tc.tile_pool(name="consts", bufs=1))
        # all 8 sequences fit in SBUF in fp16 (4 MiB) — prefetch everything
        # upfront so the input DMA stream runs back-to-back at full rate
        xpool = ctx.enter_context(tc.tile_pool(name="xin", bufs=_S))
        xtpool = ctx.enter_context(tc.tile_pool(name="xt", bufs=2))
        ypool = ctx.enter_context(tc.tile_pool(name="yout", bufs=2))
        ptp = ctx.enter_context(tc.tile_pool(name="pt", bufs=4, space="PSUM"))
        pyp = ctx.enter_context(tc.tile_pool(name="py", bufs=4, space="PSUM"))

        id_sb = cpool.tile([_P, _P], iodt)
        nc.sync.dma_start(id_sb[:], id_d.ap())
        coef_sb = cpool.tile([_P, NC], iodt)
        nc.sync.dma_start(coef_sb[:], coef_d.ap())
        if mode == "f32r":
            coef_c = cpool.tile([_P, NC], cdt)
            nc.vector.tensor_copy(coef_c[:], coef_sb[:])
        else:
            coef_c = coef_sb

        def load_stage(s):
            # load x[s] as [128, 2048] in two half-loads so the first
            # transpose group starts after ~half the transfer
            xs = xpool.tile([_P, _F], iodt)
            nc.sync.dma_start(xs[:, 0 : _F // 2], x_d.ap()[s][:, 0 : _F // 2])
            nc.sync.dma_start(xs[:, _F // 2 : _F], x_d.ap()[s][:, _F // 2 : _F])
            return xs

        def transpose_stage(xs):
            # PE transposes, 8 per PSUM bank (fp16: 2 KiB/partition).
            xt = xtpool.tile([_P, XT_SLOTS * _P], cdt)
            # element-indexed view for the m1 boundary ops (gpsimd can't
            # address float32r; fp16/f32 are fine natively)
            xt32 = xt[:].bitcast(f32) if mode == "f32r" else xt[:]
            for gidx in range(2):
                ptile = ptp.tile([_P, 8 * _P], iodt)
                for q in range(8):
                    f1 = gidx * 8 + q
                    nc.tensor.transpose(
                        ptile[:, q * _P : (q + 1) * _P],
                        xs[:, f1 * _P : (f1 + 1) * _P],
                        id_sb[:],
                    )
                # dense contiguous evacuation (gpsimd cannot read PSUM)
                eng = nc.vector.tensor_copy if gidx == 0 else nc.scalar.copy
                eng(
                    xt[:, gidx * 8 * _P : (gidx + 1) * 8 * _P],
                    ptile[:, 0 : 8 * _P],
                )

            # m1 boundary tile: m1[col p] = tile15[col p-1], col 0 = 0
            m1 = _NT * _P
            nc.gpsimd.memset(xt32[:, m1 : m1 + 1], 0.0)
            nc.gpsimd.tensor_copy(
                xt32[:, m1 + 1 : m1 + _P],
                xt32[:, 15 * _P : 16 * _P - 1],
            )
            return xt

        def fir_stage(s, xt):
            # FIR matmuls, PSUM-accumulated B-half, dense evacuation.
            # Per output tile c: psum[c] = xt[c] @ A (start) then += xt[c-1]
            # @ B (accumulate).  Tiles descend 15..0 so consecutive matmuls
            # share stationary operands; groups of 4 tiles per PSUM bank.
            ys = ypool.tile([_P, _F], iodt)
            evac = [nc.scalar.copy, nc.vector.tensor_copy] * 2
            for gi, hi in enumerate([15, 11, 7, 3]):
                lo = hi - 3
                pt_ = pyp.tile([_P, 4 * _P], f32, tag="py")
                for c in range(hi, lo - 1, -1):
                    col = (c - lo) * _P
                    nc.tensor.matmul(
                        pt_[:, col : col + _P],
                        xt[:, c * _P : (c + 1) * _P],
                        coef_c[:, 0:_P],
                        start=True,
                        stop=False,
                    )
                    prev = _NT if c == 0 else c - 1   # m1 tile for c == 0
                    nc.tensor.matmul(
                        pt_[:, col : col + kb],
                        xt[:, prev * _P : (prev + 1) * _P],
                        coef_c[:, _P : _P + kb],
                        start=False,
                        stop=True,
                    )
                evac[gi](
                    ys[:, lo * _P : (hi + 1) * _P],
                    pt_[:, 0 : 4 * _P],
                )
                # stream each finished half of y out immediately, on
                # separate queues, so the output DMA doesn't trail compute
                if hi == 11:
                    nc.gpsimd.dma_start(
                        y_d.ap()[s][:, _F // 2 : _F], ys[:, _F // 2 : _F]
                    )
                elif hi == 3:
                    nc.sync.dma_start(
                        y_d.ap()[s][:, 0 : _F // 2], ys[:, 0 : _F // 2]
                    )

        from contextlib import nullcontext
        loop_ctx = tc.For_i(0, repeat, 1) if repeat > 1 else nullcontext()
        with loop_ctx:
            # All input DMAs issue upfront (back-to-back on the queue);
            # the PE stream is software-pipelined across sequences: the
            # transposes of sequence s+1 are emitted BEFORE the FIR
            # matmuls of sequence s, so the PE never stalls on the
            # cross-engine xt evacuation round-trip (stalls reset the PE
            # DVFS ramp to half clock).
            xss = [load_stage(s) for s in range(_S)]
            xts = [None] * _S
            xts[0] = transpose_stage(xss[0])
            for s in range(_S):
                if s + 1 < _S:
                    xts[s + 1] = transpose_stage(xss[s + 1])
                fir_stage(s, xts[s])
                xts[s] = None

    nc.compile()
    return nc


def _make_runner(mode, kb, repeat=1):
    """Compile the bass program and wrap it in a cached shard_map'd jit."""
    import jax
    import numpy as _np
    from jax.sharding import Mesh, PartitionSpec
    from jax.experimental.shard_map import shard_map
    from concourse import bass2jax, mybir

    nc = _build_program(mode, kb, repeat)

    io_np = np.float16 if mode == "fp16" else np.float32

    if os.environ.get("BIQUAD_SIM") == "1":
        def run_sim(x_all, coef):
            from concourse import bass_interp
            y_all = np.zeros_like(x_all)
            ident = np.eye(_P, dtype=io_np)
            ncs = int(os.environ.get("BIQUAD_SIM_CORES", str(_NCORES)))
            for c in range(ncs):
                sim = bass_interp.CoreSim(nc)
                sim.tensor("x")[:] = x_all[c * _S : (c + 1) * _S]
                sim.tensor("coef")[:] = coef
                sim.tensor("ident")[:] = ident
                sim.simulate()
                y_all[c * _S : (c + 1) * _S] = sim.tensor("y")
            return y_all
        return run_sim

    bass2jax.install_neuronx_cc_hook()

    partition_name = (
        nc.partition_id_tensor.name if nc.partition_id_tensor else None
    )
    in_names, out_names, out_avals = [], [], []
    for alloc in nc.m.functions[0].allocations:
        if not isinstance(alloc, mybir.MemoryLocationSet):
            continue
        name = alloc.memorylocations[0].name
        if alloc.kind == "ExternalInput":
            if name != partition_name:
                in_names.append(name)
        elif alloc.kind == "ExternalOutput":
            out_names.append(name)
            out_avals.append(
                jax.core.ShapedArray(
                    tuple(alloc.tensor_shape), mybir.dt.np(alloc.dtype)
                )
            )
    n_params = len(in_names)
    in_names.extend(out_names)
    if partition_name is not None:
        in_names.append(partition_name)

    def _body(*args):
        operands = list(args)
        if partition_name is not None:
            operands.append(bass2jax.partition_id_tensor())
        outs = bass2jax._bass_exec_p.bind(
            *operands,
            out_avals=tuple(out_avals),
            in_names=tuple(in_names),
            out_names=tuple(out_names),
            lowering_input_output_aliases=(),
            sim_require_finite=True,
            sim_require_nnan=True,
            nc=nc,
        )
        return tuple(outs)

    devices = jax.devices()[:_NCORES]
    mesh = Mesh(_np.asarray(devices), ("core",))
    n_outs = len(out_names)
    in_specs = (PartitionSpec("core"),) * (n_params + n_outs)
    out_specs = (PartitionSpec("core"),) * n_outs
    sharded = jax.jit(
        shard_map(
            _body, mesh=mesh, in_specs=in_specs, out_specs=out_specs,
            check_rep=False,
        ),
        keep_unused=True,
    )

    name_to_idx = {n: i for i, n in enumerate(in_names[:n_params])}
    ident = np.eye(_P, dtype=io_np)

    def run_hw(x_all, coef):
        # x_all: [64, 128, 2048] fp32; returns y_all same shape
        per_core_ins = {
            "x": x_all.reshape(_NCORES * _S, _P, _F),
            "coef": np.concatenate([coef] * _NCORES, axis=0),
            "ident": np.concatenate([ident] * _NCORES, axis=0),
        }
        args = [None] * n_params
        for n, i in name_to_idx.items():
            args[i] = per_core_ins[n]
        zeros = [
            np.zeros((_NCORES * a.shape[0], *a.shape[1:]), a.dtype)
            for a in out_avals
        ]
        outs = sharded(*args, *zeros)
        y_idx = out_names.index("y")
        return np.asarray(outs[y_idx]).reshape(_B, _P, _F)

    run_hw.sharded = sharded
    run_hw.meta = (in_names, out_names, out_avals, n_params, name_to_idx, ident)
    run_hw.nc = nc

    def make_chain():
        """Jit that runs the kernel k (runtime scalar) times back-to-back on
        device, feeding y back as x — for timing (marginal cost per step ≈
        one on-device execution). fori_loop keeps the bass_exec custom call
        appearing exactly once in the module (hook limitation), and a
        runtime k avoids recompiling per chain length."""
        x_idx = name_to_idx["x"]
        y_idx = out_names.index("y")

        def chained(k, *args):
            args = list(args)

            def body(_, x):
                a = list(args)
                a[x_idx] = x
                return _body(*a)[y_idx]

            y = jax.lax.fori_loop(0, k, body, args[x_idx])
            return (y,)

        return jax.jit(
            shard_map(
                chained, mesh=mesh,
                in_specs=(PartitionSpec(),) + in_specs,
                out_specs=(PartitionSpec("core"),),
                check_rep=False,
            ),
            keep_unused=True,
        )

    run_hw.make_chain = make_chain
    return run_hw


def _get_runner(mode, kb, repeat=1):
    key = (mode, kb, repeat, os.environ.get("BIQUAD_SIM") == "1")
    if key not in _runner_cache:
        _runner_cache[key] = _make_runner(mode, kb, repeat)
    return _runner_cache[key]


def _prepare(b0, b1, b2, a1, a2):
    """Impulse response, truncation length, coefficient block."""
    g = _impulse_response(b0, b1, b2, a1, a2, 2 * _P)
    mag = np.abs(g)
    scale = mag.max() + 1e-300
    sig = np.nonzero(mag > 1e-9 * scale)[0]
    K = int(sig[-1]) + 1 if len(sig) else 1
    if K > _P:
        raise ValueError(
            f"impulse response needs {K} taps (> {_P}); filter too close "
            "to instability for the truncated-FIR kernel"
        )
    kb = max(32, ((K + 15) // 16) * 16)   # B-half width, 16-col aligned
    if _MODE == "f32r":
        kb = _P                            # keep N >= 256 for full-rate f32r
    coef = _coef_block(g[: _P + kb], kb)
    return coef, kb


def kernel(x, b0, b1, b2, a1, a2):
    assert x.shape == (_B, _T, 1), x.shape
    coef, kb = _prepare(
        float(b0[0]), float(b1[0]), float(b2[0]), float(a1[0]), float(a2[0])
    )
    run = _get_runner(_MODE, kb)
    io_np = np.float16 if _MODE == "fp16" else np.float32
    x_all = np.ascontiguousarray(x, dtype=io_np).reshape(_B, _P, _F)
    y_all = run(x_all, coef.astype(io_np))
    return y_all.reshape(_B, _T, 1).astype(np.float32)

